# revision 4
# baseline (speedup 1.0000x reference)
"""Trainium2 Bass kernel for nn_A100SimilarityCorrector.

Full inputs in, full output out. Data-parallel over the batch: 8 batch
elements -> 8 NeuronCores, one [512,512] similarity slice per core.

Key structure: in eval mode the 4-layer MLP acts independently on each
scalar similarity x, so it collapses to a 1-D function f(x) = sigmoid(g(x))
with g piecewise-linear. BatchNorms fold exactly into the weights; g is
fit on the host with a low-degree polynomial p (exact linear for the
default zero-bias weights), and the device evaluates
    out = (1-I) o (Z + Z^T),  Z = 0.5*rw*(m m^T) o sigmoid(p(A)) + 0.5*(1-rw)*A
with ScalarE (sigmoid), VectorE (masking/residual), TensorE (128x128
block transposes for the symmetrization) and DMA in/out.
"""
import sys

sys.path.insert(0, "/opt/trn_rl_repo")

import numpy as np
import ml_dtypes

EPS = 1e-5
B, N, P = 8, 512, 128
NT = N // P  # 4 row tiles per core


def _fit_scalar_fn(w1, b1, W2, b2, g1, be1, m1, v1, g2, be2, m2, v2,
                   W3, b3, W4, b4, xlo, xhi):
    """Fold BN into weights, then least-squares fit poly p with
    sigmoid(p(x)) ~ f(x) on [xlo, xhi]. Returns (coeffs lowest-first, max_err)."""
    f64 = np.float64
    w1 = w1.astype(f64); b1 = b1.astype(f64)
    s1 = g1.astype(f64) / np.sqrt(v1.astype(f64) + EPS)
    t1 = be1.astype(f64) - m1.astype(f64) * s1
    W2p = s1[:, None] * W2.astype(f64)
    b2p = b2.astype(f64) + t1 @ W2.astype(f64)
    s2 = g2.astype(f64) / np.sqrt(v2.astype(f64) + EPS)
    t2 = be2.astype(f64) - m2.astype(f64) * s2
    W3p = s2[:, None] * W3.astype(f64)
    b3p = b3.astype(f64) + t2 @ W3.astype(f64)
    W4 = W4.astype(f64); b4 = b4.astype(f64)

    def g(x):
        h = np.maximum(x[:, None] * w1 + b1, 0.0)
        h = np.maximum(h @ W2p + b2p, 0.0)
        h = np.maximum(h @ W3p + b3p, 0.0)
        return (h @ W4 + b4)[:, 0]

    pad = 0.02 * max(xhi - xlo, 1e-3)
    xs = np.linspace(xlo - pad, xhi + pad, 4097)
    gx = g(xs)
    fx = 1.0 / (1.0 + np.exp(-gx))
    # weight the fit of p~g by sigmoid'(g): err through sigmoid ~ w*(p-g)
    w = fx * (1.0 - fx) + 1e-3
    best = None
    for d in range(1, 9):
        V = np.vander(xs, d + 1, increasing=True)
        coef, *_ = np.linalg.lstsq(V * w[:, None], gx * w, rcond=None)
        fit = 1.0 / (1.0 + np.exp(-(V @ coef)))
        err = np.abs(fit - fx).max()
        if best is None or err < best[1]:
            best = (coef, err)
        if err < 1.5e-3:
            break
    return best


def _build_program(coef, rw):
    """Build the SPMD Bacc program (one core's view). Returns finalized nc."""
    import concourse.bass as bass
    import concourse.bacc as bacc
    import concourse.mybir as mybir
    from concourse.tile import TileContext

    dt = mybir.dt
    ALU = mybir.AluOpType
    ACTF = mybir.ActivationFunctionType

    crw = 0.5 * rw          # scale on the corrected (masked sigmoid) part
    ca = 0.5 * (1.0 - rw)   # scale on the residual part
    d = len(coef) - 1

    nc = bacc.Bacc()
    a = nc.declare_dram_parameter("a", [N, N], dt.float32, isOutput=False)
    cpack = nc.declare_dram_parameter("cpack", [P, N + 2 * P], dt.bfloat16,
                                      isOutput=False)  # mcol | diagm | ident
    mpart = nc.declare_dram_parameter("mpart", [P, NT], dt.float32, isOutput=False)
    out = nc.declare_dram_parameter("out", [N, N], dt.float32, isOutput=True)

    with TileContext(nc) as tc:
        with (
            tc.tile_pool(name="sb", bufs=1) as sb,
            tc.tile_pool(name="ps", bufs=1, space="PSUM") as ps,
        ):
            a_sb = sb.tile([P, NT * N], dt.float32, name="a_sb")
            abf = sb.tile([P, NT * N], dt.bfloat16, name="abf")
            F = sb.tile([P, NT * N], dt.bfloat16, name="F")
            Z = sb.tile([P, NT * N], dt.bfloat16, name="Z")
            osb = sb.tile([P, NT * N], dt.float32, name="osb")
            cpack_sb = sb.tile([P, N + 2 * P], dt.bfloat16, name="cpack_sb")
            mpart_sb = sb.tile([P, NT], dt.float32, name="mpart_sb")
            mcol_sb = cpack_sb[:, 0:N]
            diag_sb = cpack_sb[:, N:N + P]
            id_sb = cpack_sb[:, N + P:N + 2 * P]
            pt = [ps.tile([P, N], dt.bfloat16, name=f"pt{r}") for r in range(NT)]
            c0t = sb.tile([P, 1], dt.float32, name="c0t")
            nc.vector.memset(c0t[:], float(coef[0]))

            nc.gpsimd.dma_start(out=cpack_sb[:], in_=cpack[:])
            nc.gpsimd.dma_start(out=mpart_sb[:], in_=mpart[:])
            # warm the sigmoid table while input DMAs run (scratch tile)
            warm = sb.tile([P, 1], dt.float32, name="warm")
            nc.vector.memset(warm[:], 0.0)
            nc.scalar.activation(warm[:], warm[:], ACTF.Sigmoid)

            nc.sync.dma_start(
                out=a_sb[:].rearrange("p (t j) -> p t j", j=N),
                in_=a.rearrange("(t p) j -> p t j", p=P))
            for t in range(NT):
                s = slice(t * N, (t + 1) * N)

                if d == 1:
                    # sigmoid(c1*a + c0) straight off the f32 input
                    nc.scalar.activation(F[:, s], a_sb[:, s], ACTF.Sigmoid,
                                         bias=c0t[:], scale=float(coef[1]))
                else:
                    nc.scalar.copy(abf[:, s], a_sb[:, s])
                    # Horner-style chain: acc = c_d*x + c_{d-1};
                    # acc = (acc + c_k)*x ...; F = sigmoid(acc + c_0)
                    nc.vector.tensor_scalar(F[:, s], abf[:, s], float(coef[d]),
                                            float(coef[d - 1]), ALU.mult, ALU.add)
                    for k in range(d - 2, 0, -1):
                        nc.vector.scalar_tensor_tensor(
                            F[:, s], F[:, s], float(coef[k]), abf[:, s],
                            ALU.add, ALU.mult)
                    nc.scalar.activation(F[:, s], F[:, s], ACTF.Sigmoid,
                                         bias=c0t[:], scale=1.0)

                # column mask (bf16 2x TT), then row mask * crw (per-partition TS)
                nc.vector.tensor_tensor(F[:, s], F[:, s], mcol_sb, ALU.mult)
                nc.vector.tensor_scalar(F[:, s], F[:, s], mpart_sb[:, t:t + 1],
                                        crw, ALU.mult, ALU.mult)
                if d == 1:
                    # abf = ca * A in bf16 (free scale on the ACT copy)
                    nc.scalar.mul(abf[:, s], a_sb[:, s], ca)
                    nc.vector.tensor_tensor(Z[:, s], abf[:, s], F[:, s], ALU.add)
                else:
                    # Z = (abf * ca) + masked F
                    nc.vector.scalar_tensor_tensor(Z[:, s], abf[:, s], ca, F[:, s],
                                                   ALU.mult, ALU.add)
                # zero the diagonal block of this row tile
                db = slice(t * N + t * P, t * N + (t + 1) * P)
                nc.vector.tensor_tensor(Z[:, db], Z[:, db], diag_sb, ALU.mult)

                # transposes of this column of blocks into each psum row tile
                for r in range(NT):
                    blk = slice(t * N + r * P, t * N + (r + 1) * P)
                    nc.tensor.transpose(pt[r][:, t * P:(t + 1) * P], Z[:, blk],
                                        id_sb)

            for r in range(NT):
                s = slice(r * N, (r + 1) * N)
                nc.vector.tensor_tensor(osb[:, s], Z[:, s], pt[r][:], ALU.add)
            nc.sync.dma_start(
                out=out.rearrange("(t p) j -> p t j", p=P),
                in_=osb[:].rearrange("p (t j) -> p t j", j=N))

    nc.finalize()
    return nc


_CACHE = {}


def kernel(similarity_matrix, node_masks, W1, b1, g1, be1, m1, v1,
           W2, b2, g2, be2, m2, v2, W3, b3, W4, b4, residual_weight):
    from concourse.bass_utils import run_bass_kernel_spmd

    sim = np.asarray(similarity_matrix, dtype=np.float32)
    masks = np.asarray(node_masks)
    assert sim.shape == (B, N, N), sim.shape
    rw = float(np.asarray(residual_weight))

    coef, fit_err = _fit_scalar_fn(
        np.asarray(W1)[0], np.asarray(b1), np.asarray(W2), np.asarray(b2),
        np.asarray(g1), np.asarray(be1), np.asarray(m1), np.asarray(v1),
        np.asarray(g2), np.asarray(be2), np.asarray(m2), np.asarray(v2),
        np.asarray(W3), np.asarray(b3), np.asarray(W4), np.asarray(b4),
        float(sim.min()), float(sim.max()))

    key = (tuple(np.round(coef, 12)), round(rw, 12))
    if key not in _CACHE:
        _CACHE[key] = _build_program(coef, rw)
    nc = _CACHE[key]

    mf = masks.astype(np.float32)
    bf16 = ml_dtypes.bfloat16
    ident = np.eye(P, dtype=np.float32)
    diagm = 1.0 - ident
    in_maps = []
    for b in range(B):
        mcol = np.broadcast_to(mf[b], (P, N))
        cpack = np.concatenate([mcol, diagm, ident], axis=1).astype(bf16)
        mpart = mf[b].reshape(NT, P).T.copy().astype(np.float32)
        in_maps.append(dict(a=sim[b], cpack=cpack, mpart=mpart))

    res = run_bass_kernel_spmd(nc, in_maps, core_ids=list(range(B)))
    out = np.stack([res.results[b]["out"] for b in range(B)], axis=0)
    return out.astype(np.float32)


# revision 5
# speedup vs baseline: 1.1442x; 1.1442x over previous
"""Trainium2 Bass kernel for nn_A100SimilarityCorrector.

Full inputs in, full output out. Data-parallel over the batch: 8 batch
elements -> 8 NeuronCores, one [512,512] similarity slice per core.

Key structure: in eval mode the 4-layer MLP acts independently on each
scalar similarity x, so it collapses to a 1-D function f(x) = sigmoid(g(x))
with g piecewise-linear. BatchNorms fold exactly into the weights; g is
fit on the host with a low-degree polynomial p (exact linear for the
default zero-bias weights), and the device evaluates
    out = (1-I) o (Z + Z^T),  Z = 0.5*rw*(m m^T) o sigmoid(p(A)) + 0.5*(1-rw)*A
with ScalarE (sigmoid), VectorE (masking/residual), TensorE (128x128
block transposes for the symmetrization) and DMA in/out.
"""
import sys

sys.path.insert(0, "/opt/trn_rl_repo")

import numpy as np
import ml_dtypes

EPS = 1e-5
B, N, P = 8, 512, 128
NT = N // P  # 4 row tiles per core


def _fit_scalar_fn(w1, b1, W2, b2, g1, be1, m1, v1, g2, be2, m2, v2,
                   W3, b3, W4, b4, xlo, xhi):
    """Fold BN into weights, then least-squares fit poly p with
    sigmoid(p(x)) ~ f(x) on [xlo, xhi]. Returns (coeffs lowest-first, max_err)."""
    f64 = np.float64
    w1 = w1.astype(f64); b1 = b1.astype(f64)
    s1 = g1.astype(f64) / np.sqrt(v1.astype(f64) + EPS)
    t1 = be1.astype(f64) - m1.astype(f64) * s1
    W2p = s1[:, None] * W2.astype(f64)
    b2p = b2.astype(f64) + t1 @ W2.astype(f64)
    s2 = g2.astype(f64) / np.sqrt(v2.astype(f64) + EPS)
    t2 = be2.astype(f64) - m2.astype(f64) * s2
    W3p = s2[:, None] * W3.astype(f64)
    b3p = b3.astype(f64) + t2 @ W3.astype(f64)
    W4 = W4.astype(f64); b4 = b4.astype(f64)

    def g(x):
        h = np.maximum(x[:, None] * w1 + b1, 0.0)
        h = np.maximum(h @ W2p + b2p, 0.0)
        h = np.maximum(h @ W3p + b3p, 0.0)
        return (h @ W4 + b4)[:, 0]

    pad = 0.02 * max(xhi - xlo, 1e-3)
    xs = np.linspace(xlo - pad, xhi + pad, 4097)
    gx = g(xs)
    fx = 1.0 / (1.0 + np.exp(-gx))
    # weight the fit of p~g by sigmoid'(g): err through sigmoid ~ w*(p-g)
    w = fx * (1.0 - fx) + 1e-3
    best = None
    for d in range(1, 9):
        V = np.vander(xs, d + 1, increasing=True)
        coef, *_ = np.linalg.lstsq(V * w[:, None], gx * w, rcond=None)
        fit = 1.0 / (1.0 + np.exp(-(V @ coef)))
        err = np.abs(fit - fx).max()
        if best is None or err < best[1]:
            best = (coef, err)
        if err < 1.5e-3:
            break
    return best


def _build_program(coef, rw):
    """Build the SPMD Bacc program (one core's view). Returns finalized nc."""
    import concourse.bass as bass
    import concourse.bacc as bacc
    import concourse.mybir as mybir
    from concourse.tile import TileContext

    dt = mybir.dt
    ALU = mybir.AluOpType
    ACTF = mybir.ActivationFunctionType

    crw = 0.5 * rw          # scale on the corrected (masked sigmoid) part
    ca = 0.5 * (1.0 - rw)   # scale on the residual part
    d = len(coef) - 1

    nc = bacc.Bacc()
    a = nc.declare_dram_parameter("a", [N, N], dt.float32, isOutput=False)
    cpack = nc.declare_dram_parameter("cpack", [P, N + 2 * P], dt.bfloat16,
                                      isOutput=False)  # mcol | diagm | ident
    mpart = nc.declare_dram_parameter("mpart", [P, NT], dt.float32, isOutput=False)
    out = nc.declare_dram_parameter("out", [N, N], dt.float32, isOutput=True)

    with TileContext(nc) as tc:
        with (
            tc.tile_pool(name="sb", bufs=1) as sb,
            tc.tile_pool(name="ps", bufs=1, space="PSUM") as ps,
        ):
            a_sb = sb.tile([P, NT * N], dt.float32, name="a_sb")
            abf = sb.tile([P, NT * N], dt.bfloat16, name="abf")
            F = sb.tile([P, NT * N], dt.bfloat16, name="F")
            Z = sb.tile([P, NT * N], dt.bfloat16, name="Z")
            osb = sb.tile([P, NT * N], dt.float32, name="osb")
            cpack_sb = sb.tile([P, N + 2 * P], dt.bfloat16, name="cpack_sb")
            mpart_sb = sb.tile([P, NT], dt.float32, name="mpart_sb")
            mcol_sb = cpack_sb[:, 0:N]
            diag_sb = cpack_sb[:, N:N + P]
            id_sb = cpack_sb[:, N + P:N + 2 * P]
            pt = [ps.tile([P, N], dt.bfloat16, name=f"pt{r}") for r in range(NT)]
            c0t = sb.tile([P, 1], dt.float32, name="c0t")
            nc.vector.memset(c0t[:], float(coef[0]))

            nc.gpsimd.dma_start(out=cpack_sb[:], in_=cpack[:])
            nc.gpsimd.dma_start(out=mpart_sb[:], in_=mpart[:])
            # warm the sigmoid table while input DMAs run (scratch tile)
            warm = sb.tile([P, 1], dt.float32, name="warm")
            nc.vector.memset(warm[:], 0.0)
            nc.scalar.activation(warm[:], warm[:], ACTF.Sigmoid)

            a3 = a.rearrange("(t p) j -> p t j", p=P)
            asb3 = a_sb[:].rearrange("p (t j) -> p t j", j=N)
            nc.sync.dma_start(out=asb3[:, 0:2, :], in_=a3[:, 0:2, :])
            nc.sync.dma_start(out=asb3[:, 2:4, :], in_=a3[:, 2:4, :])
            for t in range(NT):
                s = slice(t * N, (t + 1) * N)

                if d == 1:
                    # sigmoid(c1*a + c0) straight off the f32 input
                    nc.scalar.activation(F[:, s], a_sb[:, s], ACTF.Sigmoid,
                                         bias=c0t[:], scale=float(coef[1]))
                else:
                    nc.scalar.copy(abf[:, s], a_sb[:, s])
                    # Horner-style chain: acc = c_d*x + c_{d-1};
                    # acc = (acc + c_k)*x ...; F = sigmoid(acc + c_0)
                    nc.vector.tensor_scalar(F[:, s], abf[:, s], float(coef[d]),
                                            float(coef[d - 1]), ALU.mult, ALU.add)
                    for k in range(d - 2, 0, -1):
                        nc.vector.scalar_tensor_tensor(
                            F[:, s], F[:, s], float(coef[k]), abf[:, s],
                            ALU.add, ALU.mult)
                    nc.scalar.activation(F[:, s], F[:, s], ACTF.Sigmoid,
                                         bias=c0t[:], scale=1.0)

                # column mask (bf16 2x TT), then row mask * crw (per-partition TS)
                nc.vector.tensor_tensor(F[:, s], F[:, s], mcol_sb, ALU.mult)
                nc.vector.tensor_scalar(F[:, s], F[:, s], mpart_sb[:, t:t + 1],
                                        crw, ALU.mult, ALU.mult)
                if d == 1:
                    # abf = ca * A in bf16 (free scale on the ACT copy)
                    nc.scalar.mul(abf[:, s], a_sb[:, s], ca)
                    nc.vector.tensor_tensor(Z[:, s], abf[:, s], F[:, s], ALU.add)
                else:
                    # Z = (abf * ca) + masked F
                    nc.vector.scalar_tensor_tensor(Z[:, s], abf[:, s], ca, F[:, s],
                                                   ALU.mult, ALU.add)
                # zero the diagonal block of this row tile
                db = slice(t * N + t * P, t * N + (t + 1) * P)
                nc.vector.tensor_tensor(Z[:, db], Z[:, db], diag_sb, ALU.mult)

                # transposes of this column of blocks into each psum row tile
                for r in range(NT):
                    blk = slice(t * N + r * P, t * N + (r + 1) * P)
                    nc.tensor.transpose(pt[r][:, t * P:(t + 1) * P], Z[:, blk],
                                        id_sb)

            for r in range(NT):
                s = slice(r * N, (r + 1) * N)
                nc.vector.tensor_tensor(osb[:, s], Z[:, s], pt[r][:], ALU.add)
                nc.sync.dma_start(out=out[r * P:(r + 1) * P, :], in_=osb[:, s])

    nc.finalize()
    return nc


_CACHE = {}


def kernel(similarity_matrix, node_masks, W1, b1, g1, be1, m1, v1,
           W2, b2, g2, be2, m2, v2, W3, b3, W4, b4, residual_weight):
    from concourse.bass_utils import run_bass_kernel_spmd

    sim = np.asarray(similarity_matrix, dtype=np.float32)
    masks = np.asarray(node_masks)
    assert sim.shape == (B, N, N), sim.shape
    rw = float(np.asarray(residual_weight))

    coef, fit_err = _fit_scalar_fn(
        np.asarray(W1)[0], np.asarray(b1), np.asarray(W2), np.asarray(b2),
        np.asarray(g1), np.asarray(be1), np.asarray(m1), np.asarray(v1),
        np.asarray(g2), np.asarray(be2), np.asarray(m2), np.asarray(v2),
        np.asarray(W3), np.asarray(b3), np.asarray(W4), np.asarray(b4),
        float(sim.min()), float(sim.max()))

    key = (tuple(np.round(coef, 12)), round(rw, 12))
    if key not in _CACHE:
        _CACHE[key] = _build_program(coef, rw)
    nc = _CACHE[key]

    mf = masks.astype(np.float32)
    bf16 = ml_dtypes.bfloat16
    ident = np.eye(P, dtype=np.float32)
    diagm = 1.0 - ident
    in_maps = []
    for b in range(B):
        mcol = np.broadcast_to(mf[b], (P, N))
        cpack = np.concatenate([mcol, diagm, ident], axis=1).astype(bf16)
        mpart = mf[b].reshape(NT, P).T.copy().astype(np.float32)
        in_maps.append(dict(a=sim[b], cpack=cpack, mpart=mpart))

    res = run_bass_kernel_spmd(nc, in_maps, core_ids=list(range(B)))
    out = np.stack([res.results[b]["out"] for b in range(B)], axis=0)
    return out.astype(np.float32)
